# revision 23
# baseline (speedup 1.0000x reference)
# Multi-head attention on 8 Trainium2 NeuronCores — data-parallel over batch.
#
# Problem: x[8,1024,768] @ w_qkv[768,2304] -> q,k,v (12 heads, d=64);
#          softmax(q k^T / 8) v ; proj w_proj[768,768] + b_proj.
# Sharding: one batch element per core (8 cores), no collectives.
#
# Per-core kernel (all matmuls bf16 on PE, f32 accumulation in PSUM):
#   0. ~40 warmup matmuls on the identity during the initial DMA wait keep
#      the PE HAM clock-gate at 8/8 so real work starts at 2.4 GHz.
#   1. x -> SBUF (per-token-tile DMA chunks), cast bf16 (DVE),
#      PE-transpose into one xT_all tile [128, 6*1024]; the 6 transposes
#      of a token tile share one PSUM bank and are evicted by a single
#      strided ScalarE copy.
#   2. w_qkv arrives in blocks ordered (k0q0 cols, v cols, rest) so the
#      first head pairs' k/q tiles can be computed immediately after the
#      transposes; the v = x @ w_qkv[:,1536:] chains then overlap the
#      first attention pair (they fill its PE idle slots).
#   3. attention per (head-pair, q-chunk): per k-tile kc the two heads'
#      scoresT[k,q] matmuls (K=64; even head on PE rows 0-63, odd on
#      64-127) write the two halves of one [128,1024] PSUM tile and run
#      CONCURRENTLY in separate PE row groups; the tile's single exp
#      makes both heads' next scores ready simultaneously, which keeps
#      the pairs paired, and the sT pool's 2 buffers give the exp chain
#      a full exp of slack -> AV accumulation on PE; the ones column
#      yields the softmax denominator free -> reciprocal_approx_fast +
#      gpsimd partition-broadcast + one DVE mul per head.  qkT lives in
#      a 6-slot ring so later pairs' qk GEMM chains WAR-wait on an older
#      attention pair — spreading that PE work into the exp-wait holes.
#      (no max-subtraction: scores are ~N(0,1), exp cannot overflow)
#   4. out = outT-major matmul with w_proj, bias added during PSUM
#      eviction, output DMA per column chunk for an early drain.
import sys
import types

import numpy as np


def _install_axon_profile_hook():
    # The NTFF profile hook normally lives in antenv.axon_hooks; this image
    # lacks it, so recreate it from the boot helper (needed only for
    # trace=True; harmless otherwise).
    try:
        import antenv.axon_hooks  # noqa: F401
        return
    except ImportError:
        pass
    try:
        import antenv
        from trn_agent_boot.trn_boot import _ntff_profile_via_ctypes

        m = types.ModuleType("antenv.axon_hooks")
        hook = _ntff_profile_via_ctypes("/opt/axon/libaxon_pjrt.so")
        m.get_axon_ntff_profile_hook = lambda: hook
        m.set_axon_ntff_profile_hook = lambda h: None
        antenv.axon_hooks = m
        sys.modules["antenv.axon_hooks"] = m
    except Exception:
        pass


N, C, H, D = 1024, 768, 12, 64
SCALE = D ** -0.5
NT = N // 128        # 8 token tiles
CT = C // 128        # 6 channel tiles
NQC = N // 512       # 2 q-chunks
E = D + 1            # per-head v width with ones column


def build_kernel():
    import concourse.bass as bass  # noqa: F401
    import concourse.mybir as mybir
    from concourse import bacc
    from concourse.tile import TileContext
    from concourse.masks import make_identity
    from contextlib import ExitStack

    F32 = mybir.dt.float32
    BF16 = mybir.dt.bfloat16
    Exp = mybir.ActivationFunctionType.Exp

    nc = bacc.Bacc()
    x_ext = nc.declare_dram_parameter("x", [N, C], F32, isOutput=False)
    wqkv_ext = nc.declare_dram_parameter("w_qkv", [C, 3 * C], F32, isOutput=False)
    wproj_ext = nc.declare_dram_parameter("w_proj", [C, C], F32, isOutput=False)
    bproj_ext = nc.declare_dram_parameter("b_proj", [C], F32, isOutput=False)
    out_ext = nc.declare_dram_parameter("out", [N, C], F32, isOutput=True)

    with TileContext(nc) as tc, ExitStack() as ctx:
        const = ctx.enter_context(tc.tile_pool(name="const", bufs=1))
        persist = ctx.enter_context(tc.tile_pool(name="persist", bufs=1))
        stage = ctx.enter_context(tc.tile_pool(name="stage", bufs=2))
        psum_mm = ctx.enter_context(tc.tile_pool(name="psum_mm", bufs=2, space="PSUM"))
        psum_sT = ctx.enter_context(tc.tile_pool(name="psum_sT", bufs=2, space="PSUM"))
        psum_av = ctx.enter_context(tc.tile_pool(name="psum_av", bufs=2, space="PSUM"))

        # identity FIRST on the gpsimd queue (b_bcast would head-of-line
        # block it behind the b_proj DMA semaphore otherwise)
        ident = const.tile([128, 128], BF16, tag="ident")
        with tc.high_priority():
            make_identity(nc, ident)
            # HAM warmup: keep the PE busy during the initial DMA wait so
            # the clock-gate reaches 8/8 before the real matmuls start.
            for w in range(40):
                wm = psum_mm.tile([128, 128], F32, tag="mm", name=f"warm{w}")
                nc.tensor.matmul(wm[:], ident[:], ident[:], start=True, stop=True)

        w_bf = [persist.tile([128, 3 * C], BF16, tag=f"wbf{k}", name=f"wbf{k}")
                for k in range(CT)]
        wp_bf = [persist.tile([128, C], BF16, tag=f"wpbf{k}", name=f"wpbf{k}")
                 for k in range(CT)]
        # xT_all[:, c*1024 + t*128 : ...] = transpose of x token tile t,
        # channel tile c  (c-major layout so one strided evict per t works)
        xT_all = persist.tile([128, CT * N], BF16, tag="xTall", name="xTall")

        def xT(kt):
            return xT_all[:, kt * N:(kt + 1) * N]

        # qkT ring of 6 slots (3 head pairs in flight): qk_tile(hp) writes
        # the slots last READ by attention(hp-3), so the chains only become
        # schedulable once that attention pair's scores are done — this
        # back-loads the qk supply so every attention phase keeps PE
        # filler work (instead of the dataflow scheduler draining it all
        # into the earliest idle slots).
        qkT_ring = [persist.tile([128, N], BF16, tag=f"qkT{m}", name=f"qkT{m}")
                    for m in range(6)]

        def qk_slot(m):
            hp, is_q = (m, 1) if m < CT else (m - CT, 0)
            return qkT_ring[(2 * hp + is_q) % 6]
        v_aug = [persist.tile([128, H * E], BF16, tag=f"vaug{m}", name=f"vaug{m}")
                 for m in range(NT)]
        outT = [persist.tile([128, N], BF16, tag=f"outT{c}", name=f"outT{c}")
                for c in range(CT)]

        # ---- load x per token-tile chunks, cast (DVE), PE-transpose into
        #      one PSUM bank per tile, single strided ScalarE evict.
        xpool_cm = tc.tile_pool(name="xpool", bufs=1)
        xpool = xpool_cm.__enter__()
        xall = xpool.tile([128, NT * C], F32, tag="xall", name="xall")
        x_src = x_ext.rearrange("(t p) c -> p t c", p=128)
        xv = xall.rearrange("p (t c) -> p t c", c=C)
        with tc.high_priority():
            for t0, t1 in ((0, 1), (1, 2), (2, 4), (4, 6), (6, 8)):
                nc.sync.dma_start(out=xv[:, t0:t1, :], in_=x_src[:, t0:t1, :])
        xTv = xT_all.rearrange("p (c n) -> p c n", n=N)
        for t in range(NT):
            xbf = stage.tile([128, C], BF16, tag="xbf", name=f"xbf{t}")
            nc.vector.tensor_copy(xbf[:], xall[:, t * C:(t + 1) * C])
            trp = psum_mm.tile([128, C], BF16, tag="mm", name=f"trp{t}")
            for c in range(CT):
                nc.tensor.transpose(trp[:, c * 128:(c + 1) * 128],
                                    xbf[:, c * 128:(c + 1) * 128], ident[:],
                                    )
            nc.scalar.copy(xTv[:, :, t * 128:(t + 1) * 128],
                           trp[:].rearrange("p (c n) -> p c n", n=128))
        xpool_cm.__exit__(None, None, None)
        expp = ctx.enter_context(tc.tile_pool(name="expp", bufs=6))
        rbp = ctx.enter_context(tc.tile_pool(name="rbp", bufs=2))

        # ---- load w_qkv by column blocks: the k/q columns of the first
        #      two head pairs first (unblocks attention(0)), then v, then
        #      the remaining k/q columns; cast on DVE ----
        wq_blocks = [(768, 256), (0, 256), (1536, 512), (2048, 256),
                     (1024, 256), (256, 256), (1280, 256), (512, 256)]
        for bi, (cs, cw) in enumerate(wq_blocks):
            wcb = stage.tile([128, CT * 512], F32, tag="wcb", name=f"wcb{bi}")
            src = wqkv_ext.rearrange("(k p) c -> p k c", p=128)[:, :, cs:cs + cw]
            nc.sync.dma_start(out=wcb[:, :CT * cw].rearrange("p (k c) -> p k c", k=CT),
                              in_=src)
            for k in range(CT):
                nc.vector.tensor_copy(w_bf[k][:, cs:cs + cw],
                                      wcb[:, k * cw:(k + 1) * cw])

        # bias broadcast for proj (deferred: only needed at the very end)
        bf32 = const.tile([1, C], F32, tag="bf32")
        nc.sync.dma_start(out=bf32[:], in_=bproj_ext[None, :])
        b_bcast = const.tile([128, C], F32, tag="b_bcast")
        nc.gpsimd.partition_broadcast(b_bcast[:], bf32[:])

        # ---- load w_proj + cast (overlaps everything) ----
        for k in range(CT):
            wpst = stage.tile([128, C], F32, tag="wpst", name=f"wpst{k}")
            nc.sync.dma_start(out=wpst[:], in_=wproj_ext[k * 128:(k + 1) * 128, :])
            nc.vector.tensor_copy(wp_bf[k][:], wpst[:])

        def v_tiles(ms):
            # v = x @ w_qkv[:,1536:] into v_aug (strided per-head, ones col)
            for m in ms:
                va = v_aug[m].rearrange("p (h e) -> p h e", e=E)
                nc.vector.memset(va[:, :, D:E], 1.0)
                for n, (cs, cw) in enumerate([(1536, 512), (2048, 256)]):
                    vps = psum_mm.tile([128, 512], F32, tag="mm",
                                       name=f"vps{m}_{n}")
                    for kt in range(CT):
                        nc.tensor.matmul(vps[:, :cw],
                                         xT(kt)[:, m * 128:(m + 1) * 128],
                                         w_bf[kt][:, cs:cs + cw],
                                         start=(kt == 0), stop=(kt == CT - 1))
                    nh = cw // D
                    nc.vector.tensor_copy(
                        va[:, n * 8:n * 8 + nh, 0:D],
                        vps[:, :cw].rearrange("p (h e) -> p h e", e=D))

        def qk_tile(m):
            dst = qk_slot(m)
            for n in range(NQC):
                qps = psum_mm.tile([128, 512], F32, tag="mm", name=f"qps{m}_{n}")
                for kt in range(CT):
                    nc.tensor.matmul(qps[:],
                                     w_bf[kt][:, m * 128:(m + 1) * 128],
                                     xT(kt)[:, n * 512:(n + 1) * 512],
                                     start=(kt == 0), stop=(kt == CT - 1))
                nc.vector.tensor_copy(dst[:, n * 512:(n + 1) * 512], qps[:])

        def attention_pair(hp):
            # Score matmuls run as row-group-CONCURRENT pairs: the even
            # head streams from SBUF partitions 0-63 (PE rows 0-63), the
            # odd head from 64-127 — per-kc the two matmuls write the two
            # halves of one [128,1024] sT tile whose single exp makes both
            # heads' next-group scores ready at the same instant (keeps
            # the pairs paired). sT pool bufs=2 gives the exp chain a full
            # exp of slack, so ScalarE never waits on the ping-pong.
            qt = qkT_ring[(2 * hp + 1) % 6]
            kt_t = qkT_ring[(2 * hp) % 6]
            NG = NT // 2
            for qc in range(NQC):
                avs, pexps = {}, []
                for par in (0, 1):
                    avs[par] = psum_av.tile([128, 512], F32, tag="av",
                                            name=f"av{hp}_{qc}_{par}")
                for g in range(NG):
                    gp = []
                    for j in range(2):
                        kc = 2 * g + j
                        sT = psum_sT.tile([128, 1024], F32, tag="sT",
                                          name=f"sT{hp}_{qc}_{kc}")
                        # scores+exp outrank the v/qk filler chains in the
                        # scheduler's ready-heap — they pace the whole
                        # attention pipeline, and ties otherwise go to the
                        # earlier-emitted filler (AV/normalize stay at
                        # normal priority: boosting them starves the
                        # filler's DVE evictions).
                        with tc.high_priority(offset=20000):
                            for par in (0, 1):
                                ro = par * D
                                nc.tensor.matmul(
                                    sT[:, par * 512:(par + 1) * 512],
                                    kt_t[ro:ro + D, kc * 128:(kc + 1) * 128],
                                    qt[ro:ro + D, qc * 512:(qc + 1) * 512],
                                    start=True, stop=True)
                            pexp = expp.tile([128, 1024], BF16, tag="pexp",
                                             name=f"pexp{hp}_{qc}_{kc}")
                            nc.scalar.activation(pexp[:], sT[:], Exp,
                                                 scale=SCALE)
                        gp.append(pexp)
                    pexps.append(gp)
                    # 2-group skew: AV(g-2) after scores(g). The first AV
                    # quad of a q-chunk WAR-waits the PREVIOUS chunk's
                    # normalize chain (av pool rotation); with a 1-group
                    # skew it sits in the in-order PE queue ahead of
                    # scores(g2) and head-of-line blocks the exp pipeline
                    # for ~2us every chunk.
                    if g >= 2:
                        for j in range(2):
                            kc = 2 * (g - 2) + j
                            for par in (0, 1):
                                h = 2 * hp + par
                                nc.tensor.matmul(
                                    avs[par][0:E, :],
                                    v_aug[kc].rearrange("p (h e) -> p h e",
                                                        e=E)[:, h, :],
                                    pexps[g - 2][j][:, par * 512:(par + 1) * 512],
                                    start=(kc == 0), stop=False)
                for gl in (NG - 2, NG - 1):
                    for j in range(2):
                        kc = 2 * gl + j
                        for par in (0, 1):
                            h = 2 * hp + par
                            nc.tensor.matmul(
                                avs[par][0:E, :],
                                v_aug[kc].rearrange("p (h e) -> p h e", e=E)[:, h, :],
                                pexps[gl][j][:, par * 512:(par + 1) * 512],
                                start=False, stop=(kc == NT - 1))
                # normalize: outT[d, q] = av[d, q] / av[64, q]
                # (denominator to SBUF first: custom-DVE ops misread PSUM)
                for par in (0, 1):
                    h, ro, av = 2 * hp + par, par * D, avs[par]
                    den = rbp.tile([1, 512], F32, tag="den", name=f"den{h}_{qc}")
                    nc.vector.tensor_copy(den[:], av[D:E, :])
                    recip = rbp.tile([1, 512], F32, tag="recip",
                                     name=f"rcp{h}_{qc}")
                    nc.vector.reciprocal_approx_fast(recip[:], den[:])
                    rb = rbp.tile([64, 512], F32, tag="rb", name=f"rb{h}_{qc}")
                    nc.gpsimd.partition_broadcast(rb[:], recip[:])
                    nc.vector.tensor_mul(
                        outT[hp][ro:ro + D, qc * 512:(qc + 1) * 512],
                        av[0:D, :], rb[:])

        # k,q tiles for the first two head pairs straight after the
        # transposes (their w columns arrive first), then the v chains —
        # which overlap attention(0) and fill its PE idle slots.
        qk_tile(CT + 0)
        qk_tile(0)
        qk_tile(CT + 1)
        qk_tile(1)
        v_tiles(range(NT))
        for hp in range(CT):
            if hp >= 2:
                qk_tile(CT + hp)   # k tile for this head pair
                qk_tile(hp)        # q tile
            attention_pair(hp)

        # ---- output projection ----
        for m in range(NT):
            ysb = stage.tile([128, C], F32, tag="ysb", name=f"ysb{m}", bufs=4)
            for n, (cs, cw) in enumerate([(0, 512), (512, 256)]):
                pools = [(psum_mm, "mm"), (psum_av, "av"), (psum_sT, "sT")]
                pp, ptag = pools[(2 * m + n) % 3]
                yps = pp.tile([128, 512], F32, tag=ptag, name=f"yps{m}_{n}")
                for kt in range(CT):
                    nc.tensor.matmul(yps[:, :cw],
                                     outT[kt][:, m * 128:(m + 1) * 128],
                                     wp_bf[kt][:, cs:cs + cw],
                                     start=(kt == 0), stop=(kt == CT - 1))
                nc.vector.tensor_add(ysb[:, cs:cs + cw], yps[:, :cw],
                                     b_bcast[:, cs:cs + cw])
                nc.sync.dma_start(out=out_ext[m * 128:(m + 1) * 128, cs:cs + cw],
                                  in_=ysb[:, cs:cs + cw])

    nc.finalize()
    return nc


_NC_CACHE = None


def kernel(x, w_qkv, w_proj, b_proj, trace=False, trace_kwargs=None):
    global _NC_CACHE
    _install_axon_profile_hook()
    from concourse.bass_utils import run_bass_kernel_spmd

    if _NC_CACHE is None:
        _NC_CACHE = build_kernel()
    nc = _NC_CACHE

    x = np.asarray(x, dtype=np.float32)
    w_qkv = np.ascontiguousarray(np.asarray(w_qkv, dtype=np.float32))
    w_proj = np.ascontiguousarray(np.asarray(w_proj, dtype=np.float32))
    b_proj = np.ascontiguousarray(np.asarray(b_proj, dtype=np.float32))
    B = x.shape[0]
    in_maps = [{
        "x": np.ascontiguousarray(x[i]),
        "w_qkv": w_qkv,
        "w_proj": w_proj,
        "b_proj": b_proj,
    } for i in range(B)]

    kwargs = {}
    if trace:
        kwargs["trace"] = True
        if trace_kwargs:
            kwargs.update(trace_kwargs)
    res = run_bass_kernel_spmd(nc, in_maps, core_ids=list(range(B)), **kwargs)
    out = np.stack([res.results[i]["out"] for i in range(B)]).astype(np.float32)
    if trace:
        return out, res
    return out


# revision 24
# speedup vs baseline: 1.1442x; 1.1442x over previous
# Multi-head attention on 8 Trainium2 NeuronCores — data-parallel over batch.
#
# Problem: x[8,1024,768] @ w_qkv[768,2304] -> q,k,v (12 heads, d=64);
#          softmax(q k^T / 8) v ; proj w_proj[768,768] + b_proj.
# Sharding: one batch element per core (8 cores), no collectives.
#
# Per-core kernel (all matmuls bf16 on PE, f32 accumulation in PSUM):
#   0. ~40 warmup matmuls on the identity during the initial DMA wait keep
#      the PE HAM clock-gate at 8/8 so real work starts at 2.4 GHz.
#   1. x -> SBUF (per-token-tile DMA chunks), cast bf16 (DVE),
#      PE-transpose into one xT_all tile [128, 6*1024]; the 6 transposes
#      of a token tile share one PSUM bank and are evicted by a single
#      strided ScalarE copy.
#   2. w_qkv arrives in blocks ordered (k0q0 cols, v cols, rest) so the
#      first head pairs' k/q tiles can be computed immediately after the
#      transposes; the v = x @ w_qkv[:,1536:] chains then overlap the
#      first attention pair (they fill its PE idle slots).
#   3. attention per (head-pair, q-chunk): per k-tile kc the two heads'
#      scoresT[k,q] matmuls (K=64; even head on PE rows 0-63, odd on
#      64-127) write the two halves of one [128,1024] PSUM tile and run
#      CONCURRENTLY in separate PE row groups; the tile's single exp
#      makes both heads' next scores ready simultaneously, which keeps
#      the pairs paired, and the sT pool's 2 buffers give the exp chain
#      a full exp of slack -> AV accumulation on PE; the ones column
#      yields the softmax denominator free -> reciprocal_approx_fast +
#      gpsimd partition-broadcast + one DVE mul per head.  qkT lives in
#      a 6-slot ring so later pairs' qk GEMM chains WAR-wait on an older
#      attention pair — spreading that PE work into the exp-wait holes.
#      (no max-subtraction: scores are ~N(0,1), exp cannot overflow)
#   4. out = outT-major matmul with w_proj, bias added during PSUM
#      eviction, output DMA per column chunk for an early drain.
import sys
import types

import numpy as np


def _install_axon_profile_hook():
    # The NTFF profile hook normally lives in antenv.axon_hooks; this image
    # lacks it, so recreate it from the boot helper (needed only for
    # trace=True; harmless otherwise).
    try:
        import antenv.axon_hooks  # noqa: F401
        return
    except ImportError:
        pass
    try:
        import antenv
        from trn_agent_boot.trn_boot import _ntff_profile_via_ctypes

        m = types.ModuleType("antenv.axon_hooks")
        hook = _ntff_profile_via_ctypes("/opt/axon/libaxon_pjrt.so")
        m.get_axon_ntff_profile_hook = lambda: hook
        m.set_axon_ntff_profile_hook = lambda h: None
        antenv.axon_hooks = m
        sys.modules["antenv.axon_hooks"] = m
    except Exception:
        pass


N, C, H, D = 1024, 768, 12, 64
SCALE = D ** -0.5
NT = N // 128        # 8 token tiles
CT = C // 128        # 6 channel tiles
NQC = N // 512       # 2 q-chunks
E = D + 1            # per-head v width with ones column


def build_kernel():
    import concourse.bass as bass  # noqa: F401
    import concourse.mybir as mybir
    from concourse import bacc
    from concourse.tile import TileContext
    from concourse.masks import make_identity
    from contextlib import ExitStack

    F32 = mybir.dt.float32
    BF16 = mybir.dt.bfloat16
    Exp = mybir.ActivationFunctionType.Exp

    nc = bacc.Bacc()
    x_ext = nc.declare_dram_parameter("x", [N, C], F32, isOutput=False)
    wqkv_ext = nc.declare_dram_parameter("w_qkv", [C, 3 * C], F32, isOutput=False)
    wproj_ext = nc.declare_dram_parameter("w_proj", [C, C], F32, isOutput=False)
    bproj_ext = nc.declare_dram_parameter("b_proj", [C], F32, isOutput=False)
    out_ext = nc.declare_dram_parameter("out", [N, C], F32, isOutput=True)

    with TileContext(nc) as tc, ExitStack() as ctx:
        const = ctx.enter_context(tc.tile_pool(name="const", bufs=1))
        persist = ctx.enter_context(tc.tile_pool(name="persist", bufs=1))
        stage = ctx.enter_context(tc.tile_pool(name="stage", bufs=2))
        psum_mm = ctx.enter_context(tc.tile_pool(name="psum_mm", bufs=2, space="PSUM"))
        psum_sT = ctx.enter_context(tc.tile_pool(name="psum_sT", bufs=2, space="PSUM"))
        psum_av = ctx.enter_context(tc.tile_pool(name="psum_av", bufs=2, space="PSUM"))

        # identity FIRST on the gpsimd queue (b_bcast would head-of-line
        # block it behind the b_proj DMA semaphore otherwise)
        ident = const.tile([128, 128], BF16, tag="ident")
        with tc.high_priority():
            make_identity(nc, ident)
            # HAM warmup: keep the PE busy during the initial DMA wait so
            # the clock-gate reaches 8/8 before the real matmuls start.
            for w in range(40):
                wm = psum_mm.tile([128, 128], F32, tag="mm", name=f"warm{w}")
                nc.tensor.matmul(wm[:], ident[:], ident[:], start=True, stop=True)

        w_bf = [persist.tile([128, 3 * C], BF16, tag=f"wbf{k}", name=f"wbf{k}")
                for k in range(CT)]
        wp_bf = [persist.tile([128, C], BF16, tag=f"wpbf{k}", name=f"wpbf{k}")
                 for k in range(CT)]
        # xT_all[:, c*1024 + t*128 : ...] = transpose of x token tile t,
        # channel tile c  (c-major layout so one strided evict per t works)
        xT_all = persist.tile([128, CT * N], BF16, tag="xTall", name="xTall")

        def xT(kt):
            return xT_all[:, kt * N:(kt + 1) * N]

        # qkT ring of 6 slots (3 head pairs in flight): qk_tile(hp) writes
        # the slots last READ by attention(hp-3), so the chains only become
        # schedulable once that attention pair's scores are done — this
        # back-loads the qk supply so every attention phase keeps PE
        # filler work (instead of the dataflow scheduler draining it all
        # into the earliest idle slots).
        qkT_ring = [persist.tile([128, N], BF16, tag=f"qkT{m}", name=f"qkT{m}")
                    for m in range(6)]

        def qk_slot(m):
            hp, is_q = (m, 1) if m < CT else (m - CT, 0)
            return qkT_ring[(2 * hp + is_q) % 6]
        v_aug = [persist.tile([128, H * E], BF16, tag=f"vaug{m}", name=f"vaug{m}")
                 for m in range(NT)]
        outT = [persist.tile([128, N], BF16, tag=f"outT{c}", name=f"outT{c}")
                for c in range(CT)]

        # ---- load x per token-tile chunks, cast (DVE), PE-transpose into
        #      one PSUM bank per tile, single strided ScalarE evict.
        xpool_cm = tc.tile_pool(name="xpool", bufs=1)
        xpool = xpool_cm.__enter__()
        xall = xpool.tile([128, NT * C], F32, tag="xall", name="xall")
        x_src = x_ext.rearrange("(t p) c -> p t c", p=128)
        xv = xall.rearrange("p (t c) -> p t c", c=C)
        with tc.high_priority():
            for t0, t1 in ((0, 1), (1, 2), (2, 4), (4, 6), (6, 8)):
                nc.sync.dma_start(out=xv[:, t0:t1, :], in_=x_src[:, t0:t1, :])
        xTv = xT_all.rearrange("p (c n) -> p c n", n=N)
        for t in range(NT):
            xbf = stage.tile([128, C], BF16, tag="xbf", name=f"xbf{t}")
            nc.vector.tensor_copy(xbf[:], xall[:, t * C:(t + 1) * C])
            trp = psum_mm.tile([128, C], BF16, tag="mm", name=f"trp{t}")
            for c in range(CT):
                nc.tensor.transpose(trp[:, c * 128:(c + 1) * 128],
                                    xbf[:, c * 128:(c + 1) * 128], ident[:],
                                    )
            nc.scalar.copy(xTv[:, :, t * 128:(t + 1) * 128],
                           trp[:].rearrange("p (c n) -> p c n", n=128))
        xpool_cm.__exit__(None, None, None)
        expp = ctx.enter_context(tc.tile_pool(name="expp", bufs=6))
        rbp = ctx.enter_context(tc.tile_pool(name="rbp", bufs=2))

        # ---- load w_qkv by column blocks: the k/q columns of the first
        #      two head pairs first (unblocks attention(0)), then v, then
        #      the remaining k/q columns; cast on DVE ----
        wq_blocks = [(768, 256), (0, 256), (1536, 512), (2048, 256),
                     (1024, 256), (256, 256), (1280, 256), (512, 256)]
        for bi, (cs, cw) in enumerate(wq_blocks):
            wcb = stage.tile([128, CT * 512], F32, tag="wcb", name=f"wcb{bi}")
            src = wqkv_ext.rearrange("(k p) c -> p k c", p=128)[:, :, cs:cs + cw]
            nc.sync.dma_start(out=wcb[:, :CT * cw].rearrange("p (k c) -> p k c", k=CT),
                              in_=src)
            for k in range(CT):
                nc.vector.tensor_copy(w_bf[k][:, cs:cs + cw],
                                      wcb[:, k * cw:(k + 1) * cw])

        # bias broadcast for proj (deferred: only needed at the very end)
        bf32 = const.tile([1, C], F32, tag="bf32")
        nc.sync.dma_start(out=bf32[:], in_=bproj_ext[None, :])
        b_bcast = const.tile([128, C], F32, tag="b_bcast")
        nc.gpsimd.partition_broadcast(b_bcast[:], bf32[:])

        # ---- load w_proj + cast (overlaps everything) ----
        for k in range(CT):
            wpst = stage.tile([128, C], F32, tag="wpst", name=f"wpst{k}")
            nc.sync.dma_start(out=wpst[:], in_=wproj_ext[k * 128:(k + 1) * 128, :])
            nc.vector.tensor_copy(wp_bf[k][:], wpst[:])

        def v_tiles(ms):
            # v = x @ w_qkv[:,1536:] into v_aug (strided per-head, ones col)
            for m in ms:
                va = v_aug[m].rearrange("p (h e) -> p h e", e=E)
                nc.vector.memset(va[:, :, D:E], 1.0)
                for n, (cs, cw) in enumerate([(1536, 512), (2048, 256)]):
                    vps = psum_mm.tile([128, 512], F32, tag="mm",
                                       name=f"vps{m}_{n}")
                    for kt in range(CT):
                        nc.tensor.matmul(vps[:, :cw],
                                         xT(kt)[:, m * 128:(m + 1) * 128],
                                         w_bf[kt][:, cs:cs + cw],
                                         start=(kt == 0), stop=(kt == CT - 1))
                    nh = cw // D
                    nc.vector.tensor_copy(
                        va[:, n * 8:n * 8 + nh, 0:D],
                        vps[:, :cw].rearrange("p (h e) -> p h e", e=D))

        def qk_tile(m):
            dst = qk_slot(m)
            for n in range(NQC):
                qps = psum_mm.tile([128, 512], F32, tag="mm", name=f"qps{m}_{n}")
                for kt in range(CT):
                    nc.tensor.matmul(qps[:],
                                     w_bf[kt][:, m * 128:(m + 1) * 128],
                                     xT(kt)[:, n * 512:(n + 1) * 512],
                                     start=(kt == 0), stop=(kt == CT - 1))
                nc.vector.tensor_copy(dst[:, n * 512:(n + 1) * 512], qps[:])

        def attention_pair(hp):
            # Score matmuls run as row-group-CONCURRENT pairs: the even
            # head streams from SBUF partitions 0-63 (PE rows 0-63), the
            # odd head from 64-127 — per-kc the two matmuls write the two
            # halves of one [128,1024] sT tile whose single exp makes both
            # heads' next-group scores ready at the same instant (keeps
            # the pairs paired). sT pool bufs=2 gives the exp chain a full
            # exp of slack, so ScalarE never waits on the ping-pong.
            qt = qkT_ring[(2 * hp + 1) % 6]
            kt_t = qkT_ring[(2 * hp) % 6]
            NG = NT // 2
            for qc in range(NQC):
                avs, pexps = {}, []
                for par in (0, 1):
                    avs[par] = psum_av.tile([128, 512], F32, tag="av",
                                            name=f"av{hp}_{qc}_{par}")
                for g in range(NG):
                    gp = []
                    for j in range(2):
                        kc = 2 * g + j
                        sT = psum_sT.tile([128, 1024], F32, tag="sT",
                                          name=f"sT{hp}_{qc}_{kc}")
                        for par in (0, 1):
                            ro = par * D
                            nc.tensor.matmul(
                                sT[:, par * 512:(par + 1) * 512],
                                kt_t[ro:ro + D, kc * 128:(kc + 1) * 128],
                                qt[ro:ro + D, qc * 512:(qc + 1) * 512],
                                start=True, stop=True)
                        pexp = expp.tile([128, 1024], BF16, tag="pexp",
                                         name=f"pexp{hp}_{qc}_{kc}")
                        nc.scalar.activation(pexp[:], sT[:], Exp, scale=SCALE)
                        gp.append(pexp)
                    pexps.append(gp)
                    # 2-group skew: AV(g-2) after scores(g). The first AV
                    # quad of a q-chunk WAR-waits the PREVIOUS chunk's
                    # normalize chain (av pool rotation); with a 1-group
                    # skew it sits in the in-order PE queue ahead of
                    # scores(g2) and head-of-line blocks the exp pipeline
                    # for ~2us every chunk.
                    if g >= 2:
                        for j in range(2):
                            kc = 2 * (g - 2) + j
                            for par in (0, 1):
                                h = 2 * hp + par
                                nc.tensor.matmul(
                                    avs[par][0:E, :],
                                    v_aug[kc].rearrange("p (h e) -> p h e",
                                                        e=E)[:, h, :],
                                    pexps[g - 2][j][:, par * 512:(par + 1) * 512],
                                    start=(kc == 0), stop=False)
                for gl in (NG - 2, NG - 1):
                    for j in range(2):
                        kc = 2 * gl + j
                        for par in (0, 1):
                            h = 2 * hp + par
                            nc.tensor.matmul(
                                avs[par][0:E, :],
                                v_aug[kc].rearrange("p (h e) -> p h e", e=E)[:, h, :],
                                pexps[gl][j][:, par * 512:(par + 1) * 512],
                                start=False, stop=(kc == NT - 1))
                # normalize: outT[d, q] = av[d, q] / av[64, q]
                # (denominator to SBUF first: custom-DVE ops misread PSUM)
                for par in (0, 1):
                    h, ro, av = 2 * hp + par, par * D, avs[par]
                    den = rbp.tile([1, 512], F32, tag="den", name=f"den{h}_{qc}")
                    nc.vector.tensor_copy(den[:], av[D:E, :])
                    recip = rbp.tile([1, 512], F32, tag="recip",
                                     name=f"rcp{h}_{qc}")
                    nc.vector.reciprocal_approx_fast(recip[:], den[:])
                    rb = rbp.tile([64, 512], F32, tag="rb", name=f"rb{h}_{qc}")
                    nc.gpsimd.partition_broadcast(rb[:], recip[:])
                    nc.vector.tensor_mul(
                        outT[hp][ro:ro + D, qc * 512:(qc + 1) * 512],
                        av[0:D, :], rb[:])

        # k,q tiles for the first two head pairs straight after the
        # transposes (their w columns arrive first), then the v chains —
        # which overlap attention(0) and fill its PE idle slots.
        qk_tile(CT + 0)
        qk_tile(0)
        qk_tile(CT + 1)
        qk_tile(1)
        v_tiles(range(NT))
        for hp in range(CT):
            if hp >= 2:
                qk_tile(CT + hp)   # k tile for this head pair
                qk_tile(hp)        # q tile
            attention_pair(hp)

        # ---- output projection ----
        for m in range(NT):
            ysb = stage.tile([128, C], F32, tag="ysb", name=f"ysb{m}", bufs=4)
            for n, (cs, cw) in enumerate([(0, 512), (512, 256)]):
                pools = [(psum_mm, "mm"), (psum_av, "av"), (psum_sT, "sT")]
                pp, ptag = pools[(2 * m + n) % 3]
                yps = pp.tile([128, 512], F32, tag=ptag, name=f"yps{m}_{n}")
                for kt in range(CT):
                    nc.tensor.matmul(yps[:, :cw],
                                     outT[kt][:, m * 128:(m + 1) * 128],
                                     wp_bf[kt][:, cs:cs + cw],
                                     start=(kt == 0), stop=(kt == CT - 1))
                nc.vector.tensor_add(ysb[:, cs:cs + cw], yps[:, :cw],
                                     b_bcast[:, cs:cs + cw])
                nc.sync.dma_start(out=out_ext[m * 128:(m + 1) * 128, cs:cs + cw],
                                  in_=ysb[:, cs:cs + cw])

    nc.finalize()
    return nc


_NC_CACHE = None


def kernel(x, w_qkv, w_proj, b_proj, trace=False, trace_kwargs=None):
    global _NC_CACHE
    _install_axon_profile_hook()
    from concourse.bass_utils import run_bass_kernel_spmd

    if _NC_CACHE is None:
        _NC_CACHE = build_kernel()
    nc = _NC_CACHE

    x = np.asarray(x, dtype=np.float32)
    w_qkv = np.ascontiguousarray(np.asarray(w_qkv, dtype=np.float32))
    w_proj = np.ascontiguousarray(np.asarray(w_proj, dtype=np.float32))
    b_proj = np.ascontiguousarray(np.asarray(b_proj, dtype=np.float32))
    B = x.shape[0]
    in_maps = [{
        "x": np.ascontiguousarray(x[i]),
        "w_qkv": w_qkv,
        "w_proj": w_proj,
        "b_proj": b_proj,
    } for i in range(B)]

    kwargs = {}
    if trace:
        kwargs["trace"] = True
        if trace_kwargs:
            kwargs.update(trace_kwargs)
    res = run_bass_kernel_spmd(nc, in_maps, core_ids=list(range(B)), **kwargs)
    out = np.stack([res.results[i]["out"] for i in range(B)]).astype(np.float32)
    if trace:
        return out, res
    return out


# revision 25
# speedup vs baseline: 1.1633x; 1.0167x over previous
# Multi-head attention on 8 Trainium2 NeuronCores — data-parallel over batch.
#
# Problem: x[8,1024,768] @ w_qkv[768,2304] -> q,k,v (12 heads, d=64);
#          softmax(q k^T / 8) v ; proj w_proj[768,768] + b_proj.
# Sharding: one batch element per core (8 cores), no collectives.
#
# Per-core kernel (all matmuls bf16 on PE, f32 accumulation in PSUM):
#   0. ~40 warmup matmuls on the identity during the initial DMA wait keep
#      the PE HAM clock-gate at 8/8 so real work starts at 2.4 GHz.
#   1. x -> SBUF (per-token-tile DMA chunks), cast bf16 (DVE),
#      PE-transpose into one xT_all tile [128, 6*1024]; the 6 transposes
#      of a token tile share one PSUM bank and are evicted by a single
#      strided ScalarE copy.
#   2. w_qkv arrives in blocks ordered (k0q0 cols, v cols, rest) so the
#      first head pairs' k/q tiles can be computed immediately after the
#      transposes; the v = x @ w_qkv[:,1536:] chains then overlap the
#      first attention pair (they fill its PE idle slots).
#   3. attention per (head-pair, q-chunk): per k-tile kc the two heads'
#      scoresT[k,q] matmuls (K=64; even head on PE rows 0-63, odd on
#      64-127) write the two halves of one [128,1024] PSUM tile and run
#      CONCURRENTLY in separate PE row groups; the tile's single exp
#      makes both heads' next scores ready simultaneously, which keeps
#      the pairs paired, and the sT pool's 2 buffers give the exp chain
#      a full exp of slack -> AV accumulation on PE; the ones column
#      yields the softmax denominator free -> reciprocal_approx_fast +
#      gpsimd partition-broadcast + one DVE mul per head.  qkT lives in
#      a 6-slot ring so later pairs' qk GEMM chains WAR-wait on an older
#      attention pair — spreading that PE work into the exp-wait holes.
#      (no max-subtraction: scores are ~N(0,1), exp cannot overflow)
#   4. out = outT-major matmul with w_proj, bias added during PSUM
#      eviction, output DMA per column chunk for an early drain.
import sys
import types

import numpy as np


def _install_axon_profile_hook():
    # The NTFF profile hook normally lives in antenv.axon_hooks; this image
    # lacks it, so recreate it from the boot helper (needed only for
    # trace=True; harmless otherwise).
    try:
        import antenv.axon_hooks  # noqa: F401
        return
    except ImportError:
        pass
    try:
        import antenv
        from trn_agent_boot.trn_boot import _ntff_profile_via_ctypes

        m = types.ModuleType("antenv.axon_hooks")
        hook = _ntff_profile_via_ctypes("/opt/axon/libaxon_pjrt.so")
        m.get_axon_ntff_profile_hook = lambda: hook
        m.set_axon_ntff_profile_hook = lambda h: None
        antenv.axon_hooks = m
        sys.modules["antenv.axon_hooks"] = m
    except Exception:
        pass


N, C, H, D = 1024, 768, 12, 64
SCALE = D ** -0.5
NT = N // 128        # 8 token tiles
CT = C // 128        # 6 channel tiles
NQC = N // 512       # 2 q-chunks
E = D + 1            # per-head v width with ones column


def build_kernel():
    import concourse.bass as bass  # noqa: F401
    import concourse.mybir as mybir
    from concourse import bacc
    from concourse.tile import TileContext
    from concourse.masks import make_identity
    from contextlib import ExitStack

    F32 = mybir.dt.float32
    BF16 = mybir.dt.bfloat16
    Exp = mybir.ActivationFunctionType.Exp

    nc = bacc.Bacc()
    x_ext = nc.declare_dram_parameter("x", [N, C], F32, isOutput=False)
    wqkv_ext = nc.declare_dram_parameter("w_qkv", [C, 3 * C], F32, isOutput=False)
    wproj_ext = nc.declare_dram_parameter("w_proj", [C, C], F32, isOutput=False)
    bproj_ext = nc.declare_dram_parameter("b_proj", [C], F32, isOutput=False)
    out_ext = nc.declare_dram_parameter("out", [N, C], F32, isOutput=True)

    with TileContext(nc) as tc, ExitStack() as ctx:
        const = ctx.enter_context(tc.tile_pool(name="const", bufs=1))
        persist = ctx.enter_context(tc.tile_pool(name="persist", bufs=1))
        stage = ctx.enter_context(tc.tile_pool(name="stage", bufs=2))
        psum_mm = ctx.enter_context(tc.tile_pool(name="psum_mm", bufs=2, space="PSUM"))
        psum_sT = ctx.enter_context(tc.tile_pool(name="psum_sT", bufs=2, space="PSUM"))
        psum_av = ctx.enter_context(tc.tile_pool(name="psum_av", bufs=2, space="PSUM"))

        # identity FIRST on the gpsimd queue (b_bcast would head-of-line
        # block it behind the b_proj DMA semaphore otherwise)
        ident = const.tile([128, 128], BF16, tag="ident")
        with tc.high_priority():
            make_identity(nc, ident)
            # HAM warmup: keep the PE busy during the initial DMA wait so
            # the clock-gate reaches 8/8 before the real matmuls start.
            for w in range(40):
                wm = psum_mm.tile([128, 128], F32, tag="mm", name=f"warm{w}")
                nc.tensor.matmul(wm[:], ident[:], ident[:], start=True, stop=True)

        w_bf = [persist.tile([128, 3 * C], BF16, tag=f"wbf{k}", name=f"wbf{k}")
                for k in range(CT)]
        wp_bf = [persist.tile([128, C], BF16, tag=f"wpbf{k}", name=f"wpbf{k}")
                 for k in range(CT)]
        # xT_all[:, c*1024 + t*128 : ...] = transpose of x token tile t,
        # channel tile c  (c-major layout so one strided evict per t works)
        xT_all = persist.tile([128, CT * N], BF16, tag="xTall", name="xTall")

        def xT(kt):
            return xT_all[:, kt * N:(kt + 1) * N]

        # qkT ring of 6 slots (3 head pairs in flight): qk_tile(hp) writes
        # the slots last READ by attention(hp-3), so the chains only become
        # schedulable once that attention pair's scores are done — this
        # back-loads the qk supply so every attention phase keeps PE
        # filler work (instead of the dataflow scheduler draining it all
        # into the earliest idle slots).
        qkT_ring = [persist.tile([128, N], BF16, tag=f"qkT{m}", name=f"qkT{m}")
                    for m in range(6)]

        def qk_slot(m):
            hp, is_q = (m, 1) if m < CT else (m - CT, 0)
            return qkT_ring[(2 * hp + is_q) % 6]
        v_aug = [persist.tile([128, H * E], BF16, tag=f"vaug{m}", name=f"vaug{m}")
                 for m in range(NT)]
        outT = [persist.tile([128, N], BF16, tag=f"outT{c}", name=f"outT{c}")
                for c in range(CT)]

        # ---- load x per token-tile chunks, cast (DVE), PE-transpose into
        #      one PSUM bank per tile, single strided ScalarE evict.
        xpool_cm = tc.tile_pool(name="xpool", bufs=1)
        xpool = xpool_cm.__enter__()
        xall = xpool.tile([128, NT * C], F32, tag="xall", name="xall")
        x_src = x_ext.rearrange("(t p) c -> p t c", p=128)
        xv = xall.rearrange("p (t c) -> p t c", c=C)
        with tc.high_priority():
            for t0, t1 in ((0, 1), (1, 2), (2, 4), (4, 6), (6, 8)):
                nc.sync.dma_start(out=xv[:, t0:t1, :], in_=x_src[:, t0:t1, :])
        xTv = xT_all.rearrange("p (c n) -> p c n", n=N)
        for t in range(NT):
            xbf = stage.tile([128, C], BF16, tag="xbf", name=f"xbf{t}")
            nc.vector.tensor_copy(xbf[:], xall[:, t * C:(t + 1) * C])
            trp = psum_mm.tile([128, C], BF16, tag="mm", name=f"trp{t}")
            for c in range(CT):
                nc.tensor.transpose(trp[:, c * 128:(c + 1) * 128],
                                    xbf[:, c * 128:(c + 1) * 128], ident[:],
                                    )
            nc.scalar.copy(xTv[:, :, t * 128:(t + 1) * 128],
                           trp[:].rearrange("p (c n) -> p c n", n=128))
        xpool_cm.__exit__(None, None, None)
        expp = ctx.enter_context(tc.tile_pool(name="expp", bufs=6))
        rbp = ctx.enter_context(tc.tile_pool(name="rbp", bufs=2))

        # ---- load w_qkv by column blocks: the k/q columns of the first
        #      two head pairs first (unblocks attention(0)), then v, then
        #      the remaining k/q columns; cast on DVE ----
        wq_blocks = [(768, 256), (0, 256), (1536, 512), (2048, 256),
                     (1024, 256), (256, 256), (1280, 256), (512, 256)]
        for bi, (cs, cw) in enumerate(wq_blocks):
            wcb = stage.tile([128, CT * 512], F32, tag="wcb", name=f"wcb{bi}")
            src = wqkv_ext.rearrange("(k p) c -> p k c", p=128)[:, :, cs:cs + cw]
            nc.sync.dma_start(out=wcb[:, :CT * cw].rearrange("p (k c) -> p k c", k=CT),
                              in_=src)
            for k in range(CT):
                nc.vector.tensor_copy(w_bf[k][:, cs:cs + cw],
                                      wcb[:, k * cw:(k + 1) * cw])

        # bias broadcast for proj (deferred: only needed at the very end)
        bf32 = const.tile([1, C], F32, tag="bf32")
        nc.sync.dma_start(out=bf32[:], in_=bproj_ext[None, :])
        b_bcast = const.tile([128, C], F32, tag="b_bcast")
        nc.gpsimd.partition_broadcast(b_bcast[:], bf32[:])

        # ---- load w_proj + cast (overlaps everything) ----
        for k in range(CT):
            wpst = stage.tile([128, C], F32, tag="wpst", name=f"wpst{k}")
            nc.sync.dma_start(out=wpst[:], in_=wproj_ext[k * 128:(k + 1) * 128, :])
            nc.vector.tensor_copy(wp_bf[k][:], wpst[:])

        def v_tiles(ms):
            # v = x @ w_qkv[:,1536:] into v_aug (strided per-head, ones col)
            for m in ms:
                va = v_aug[m].rearrange("p (h e) -> p h e", e=E)
                nc.vector.memset(va[:, :, D:E], 1.0)
                for n, (cs, cw) in enumerate([(1536, 512), (2048, 256)]):
                    vps = psum_mm.tile([128, 512], F32, tag="mm",
                                       name=f"vps{m}_{n}")
                    for kt in range(CT):
                        nc.tensor.matmul(vps[:, :cw],
                                         xT(kt)[:, m * 128:(m + 1) * 128],
                                         w_bf[kt][:, cs:cs + cw],
                                         start=(kt == 0), stop=(kt == CT - 1))
                    nh = cw // D
                    nc.vector.tensor_copy(
                        va[:, n * 8:n * 8 + nh, 0:D],
                        vps[:, :cw].rearrange("p (h e) -> p h e", e=D))

        def qk_tile(m):
            dst = qk_slot(m)
            for n in range(NQC):
                qps = psum_mm.tile([128, 512], F32, tag="mm", name=f"qps{m}_{n}")
                for kt in range(CT):
                    nc.tensor.matmul(qps[:],
                                     w_bf[kt][:, m * 128:(m + 1) * 128],
                                     xT(kt)[:, n * 512:(n + 1) * 512],
                                     start=(kt == 0), stop=(kt == CT - 1))
                nc.vector.tensor_copy(dst[:, n * 512:(n + 1) * 512], qps[:])

        def attention_pair(hp):
            # Score matmuls run as row-group-CONCURRENT pairs: the even
            # head streams from SBUF partitions 0-63 (PE rows 0-63), the
            # odd head from 64-127 — per-kc the two matmuls write the two
            # halves of one [128,1024] sT tile whose single exp makes both
            # heads' next-group scores ready at the same instant (keeps
            # the pairs paired). sT pool bufs=2 gives the exp chain a full
            # exp of slack, so ScalarE never waits on the ping-pong.
            qt = qkT_ring[(2 * hp + 1) % 6]
            kt_t = qkT_ring[(2 * hp) % 6]
            NG = NT // 2
            NU = NQC * NG     # flat (qc, g) units; AV trails scores by 2
            avs, pexps = {}, []

            def emit_av(u):
                # AV quad for flat unit u; 2-group skew ACROSS the q-chunk
                # boundary: the first AV quad of a chunk WAR-waits the
                # previous chunk's normalize chain (av pool rotation), and
                # with a shallower skew it would sit in the in-order PE
                # queue ahead of the next scores and head-of-line block
                # the exp pipeline for ~2us every chunk.
                qcu, kc0 = u // NG, 2 * (u % NG)
                for j in range(2):
                    kc = kc0 + j
                    for par in (0, 1):
                        h = 2 * hp + par
                        nc.tensor.matmul(
                            avs[(qcu, par)][0:E, :],
                            v_aug[kc].rearrange("p (h e) -> p h e", e=E)[:, h, :],
                            pexps[u][j][:, par * 512:(par + 1) * 512],
                            start=(kc == 0), stop=(kc == NT - 1))

            def emit_norm(qc):
                # normalize: outT[d, q] = av[d, q] / av[64, q]
                # (denominator to SBUF first: custom-DVE ops misread PSUM)
                for par in (0, 1):
                    h, ro, av = 2 * hp + par, par * D, avs[(qc, par)]
                    den = rbp.tile([1, 512], F32, tag="den", name=f"den{h}_{qc}")
                    nc.vector.tensor_copy(den[:], av[D:E, :])
                    recip = rbp.tile([1, 512], F32, tag="recip",
                                     name=f"rcp{h}_{qc}")
                    nc.vector.reciprocal_approx_fast(recip[:], den[:])
                    rb = rbp.tile([64, 512], F32, tag="rb", name=f"rb{h}_{qc}")
                    nc.gpsimd.partition_broadcast(rb[:], recip[:])
                    nc.vector.tensor_mul(
                        outT[hp][ro:ro + D, qc * 512:(qc + 1) * 512],
                        av[0:D, :], rb[:])

            for u in range(NU):
                qc, g = u // NG, u % NG
                if g == 0:
                    for par in (0, 1):
                        avs[(qc, par)] = psum_av.tile([128, 512], F32, tag="av",
                                                      name=f"av{hp}_{qc}_{par}")
                gp = []
                for j in range(2):
                    kc = 2 * g + j
                    sT = psum_sT.tile([128, 1024], F32, tag="sT",
                                      name=f"sT{hp}_{qc}_{kc}")
                    for par in (0, 1):
                        ro = par * D
                        nc.tensor.matmul(
                            sT[:, par * 512:(par + 1) * 512],
                            kt_t[ro:ro + D, kc * 128:(kc + 1) * 128],
                            qt[ro:ro + D, qc * 512:(qc + 1) * 512],
                            start=True, stop=True)
                    pexp = expp.tile([128, 1024], BF16, tag="pexp",
                                     name=f"pexp{hp}_{qc}_{kc}")
                    nc.scalar.activation(pexp[:], sT[:], Exp, scale=SCALE)
                    gp.append(pexp)
                pexps.append(gp)
                if u >= 2:
                    emit_av(u - 2)
                    if (u - 2) % NG == NG - 1:
                        emit_norm((u - 2) // NG)
            for u in (NU - 2, NU - 1):
                emit_av(u)
                if u % NG == NG - 1:
                    emit_norm(u // NG)

        # k,q tiles for the first two head pairs straight after the
        # transposes (their w columns arrive first), then the v chains —
        # which overlap attention(0) and fill its PE idle slots.
        qk_tile(CT + 0)
        qk_tile(0)
        qk_tile(CT + 1)
        qk_tile(1)
        v_tiles(range(NT))
        for hp in range(CT):
            if hp >= 2:
                qk_tile(CT + hp)   # k tile for this head pair
                qk_tile(hp)        # q tile
            attention_pair(hp)

        # ---- output projection ----
        for m in range(NT):
            ysb = stage.tile([128, C], F32, tag="ysb", name=f"ysb{m}", bufs=4)
            for n, (cs, cw) in enumerate([(0, 512), (512, 256)]):
                pools = [(psum_mm, "mm"), (psum_av, "av"), (psum_sT, "sT")]
                pp, ptag = pools[(2 * m + n) % 3]
                yps = pp.tile([128, 512], F32, tag=ptag, name=f"yps{m}_{n}")
                for kt in range(CT):
                    nc.tensor.matmul(yps[:, :cw],
                                     outT[kt][:, m * 128:(m + 1) * 128],
                                     wp_bf[kt][:, cs:cs + cw],
                                     start=(kt == 0), stop=(kt == CT - 1))
                nc.vector.tensor_add(ysb[:, cs:cs + cw], yps[:, :cw],
                                     b_bcast[:, cs:cs + cw])
                nc.sync.dma_start(out=out_ext[m * 128:(m + 1) * 128, cs:cs + cw],
                                  in_=ysb[:, cs:cs + cw])

    nc.finalize()
    return nc


_NC_CACHE = None


def kernel(x, w_qkv, w_proj, b_proj, trace=False, trace_kwargs=None):
    global _NC_CACHE
    _install_axon_profile_hook()
    from concourse.bass_utils import run_bass_kernel_spmd

    if _NC_CACHE is None:
        _NC_CACHE = build_kernel()
    nc = _NC_CACHE

    x = np.asarray(x, dtype=np.float32)
    w_qkv = np.ascontiguousarray(np.asarray(w_qkv, dtype=np.float32))
    w_proj = np.ascontiguousarray(np.asarray(w_proj, dtype=np.float32))
    b_proj = np.ascontiguousarray(np.asarray(b_proj, dtype=np.float32))
    B = x.shape[0]
    in_maps = [{
        "x": np.ascontiguousarray(x[i]),
        "w_qkv": w_qkv,
        "w_proj": w_proj,
        "b_proj": b_proj,
    } for i in range(B)]

    kwargs = {}
    if trace:
        kwargs["trace"] = True
        if trace_kwargs:
            kwargs.update(trace_kwargs)
    res = run_bass_kernel_spmd(nc, in_maps, core_ids=list(range(B)), **kwargs)
    out = np.stack([res.results[i]["out"] for i in range(B)]).astype(np.float32)
    if trace:
        return out, res
    return out
